# revision 37
# baseline (speedup 1.0000x reference)
"""Multi-head-attention (single-head, no scaling) Bass kernel for 8 trn2 cores.

v2: distributed K/V projections + AllGather.

Sharding: core c owns q rows [c*B, (c+1)*B) AND k/v rows [c*B, (c+1)*B).
Weights replicated. Each core:
  A)  wk transpose (PE, f32 -> rounded f32r copyback), own-k transpose,
      kpT_own = (wk @ k_own.T + b) [D, B] f32r -> DRAM -> AllGather (CC queue)
      wq transpose, qT, qpT (overlaps the AllGather)
  C)  wv (fp16) transpose, own-v cast+transpose, vp_own [B, D] fp16
      (+bias) -> DRAM -> AllGather; wo (fp16) transpose
  B1) for each 512-panel: DMA gathered kpT panel, DVE re-round to f32r,
      scores matmuls -> resident [B, N] f32
  B2) per 128-row tile: softmax (DVE/ACT, fused exp+rowsum), DMA attn out,
      cast fp16 + PE transpose -> attnT
  Dv) DMA gathered vp -> SBUF fp16
  D)  x1 = attn @ vp, transpose, x = x1 @ wo.T + b -> DMA out
"""

import numpy as np

_DIRECT_F32R = True

try:
    import concourse.bass  # noqa: F401
except ImportError:
    import sys
    for _p in ("/opt/trn_rl_repo", "/root/.axon_site/_ro/trn_rl_repo"):
        if _p not in sys.path:
            sys.path.append(_p)

import concourse.bacc as bacc
import concourse.mybir as mybir
from concourse.bass_utils import run_bass_kernel_spmd
from concourse.tile import TileContext
from concourse.masks import make_identity

P = 128
N_CORES = 8
N_LOCAL = 0   # head panels computed locally on every core

f32 = mybir.dt.float32
f32r = mybir.dt.float32r
f16 = mybir.dt.float16
EXP = mybir.ActivationFunctionType.Exp
COPY = mybir.ActivationFunctionType.Copy
IDENT = mybir.ActivationFunctionType.Identity


def build(N=4096, D=1024, n_local=N_LOCAL):
    B = N // N_CORES            # q/k/v rows per core
    DC = D // P                 # contraction chunks (d)
    IT = B // P                 # 128-row tiles per core slice
    PW = B                      # scores panel width == per-core slice
    LPAN = N // PW
    LT = N // P
    FS = min(512, D)
    MS = D // FS
    LHEAD = n_local * B         # head rows computed locally on every core

    nc = bacc.Bacc("TRN2", target_bir_lowering=False, debug=False,
                   num_devices=N_CORES)

    q_d = nc.dram_tensor("q", [B, D], f32, kind="ExternalInput")
    k_d = nc.dram_tensor("k", [B, D], f32, kind="ExternalInput")
    v_d = nc.dram_tensor("v", [B, D], f32, kind="ExternalInput")
    if n_local:
        kh_d = nc.dram_tensor("k_head", [LHEAD, D], f32, kind="ExternalInput")
    w_d = {}
    b_d = {}
    for nm in ("wq", "wk", "wv", "wo"):
        w_d[nm] = nc.dram_tensor(nm + "_w", [D, D], f32, kind="ExternalInput")
        b_d[nm] = nc.dram_tensor(nm + "_b", [D], f32, kind="ExternalInput")
    x_out = nc.dram_tensor("x_out", [B, D], f32, kind="ExternalOutput")
    a_out = nc.dram_tensor("attn_out", [B, N], f32, kind="ExternalOutput")

    with TileContext(nc) as tc:
        const_cm = tc.tile_pool(name="const", bufs=1)
        const = const_cm.__enter__()
        dram_cm = tc.tile_pool(name="dram", bufs=1, space="DRAM")
        dram = dram_cm.__enter__()

        kp_in = dram.tile([D, B], f32, tag="kp_in")
        JH = D // 2             # feature-half split of the kp gather
        kp_all_j = [dram.tile([N_CORES, JH, B], f32, tag=f"kp_all{j}",
                              name=f"kp_all{j}") for j in range(2)]
        vp_in = dram.tile([B, D], f16, tag="vp_in")
        vp_all = dram.tile([N_CORES, B, D], f16, tag="vp_all")
        warm_in = dram.tile([1, 32], f32, tag="warm_in")
        warm_out = dram.tile([N_CORES, 32], f32, tag="warm_out")

        # tiny warm-up collective: absorbs comm-init/rendezvous cost while
        # the input DMAs and weight transposes run
        warm_sb = const.tile([1, 32], f32, tag="warm")
        nc.gpsimd.memset(warm_sb[:], 0.0)
        nc.gpsimd.dma_start(warm_in[:], warm_sb[:])
        nc.gpsimd.collective_compute(
            "AllGather", mybir.AluOpType.bypass,
            replica_groups=[list(range(N_CORES))],
            ins=[warm_in[:].opt()], outs=[warm_out[:].opt()])

        ident32 = const.tile([P, P], f32, tag="id32")
        make_identity(nc, ident32[:])
        ident16 = const.tile([P, P], f16, tag="id16")
        make_identity(nc, ident16[:])

        qb_col = const.tile([P, DC], f32, tag="qbcol")
        kb_col = const.tile([P, DC], f32, tag="kbcol")
        for jt in range(DC):
            nc.sync.dma_start(qb_col[:, jt:jt + 1],
                              b_d["wq"][jt * P:(jt + 1) * P])
            nc.sync.dma_start(kb_col[:, jt:jt + 1],
                              b_d["wk"][jt * P:(jt + 1) * P])
        vb_bc = const.tile([P, D], f16, tag="vbbc")
        ob_bc = const.tile([P, D], f16, tag="obbc")
        with tc.tile_pool(name="btmp", bufs=2) as btmp:
            for bc_t, bnm in ((vb_bc, "wv"), (ob_bc, "wo")):
                tmp = btmp.tile([P, D], f32, tag="btmp")
                nc.sync.dma_start(
                    tmp[:],
                    b_d[bnm].rearrange("(o d) -> o d", o=1).partition_broadcast(P))
                nc.vector.tensor_copy(bc_t[:], tmp[:])

        def transpose_weight(nm, wT, sb, ps, rnd):
            """DMA weight natural, PE-transpose 128x128 blocks into wT."""
            for jt in range(DC):
                wnat = sb.tile([P, D], f32, tag="wnat")
                nc.sync.dma_start(wnat[:], w_d[nm][jt * P:(jt + 1) * P, :])
                for dt in range(DC):
                    pt = ps.tile([P, P], f32, tag="tpw")
                    nc.tensor.transpose(pt[:], wnat[:, dt * P:(dt + 1) * P],
                                        ident32[:])
                    dst = wT[:, dt, jt * P:(jt + 1) * P]
                    if rnd is f32r:
                        nc.scalar.activation(dst.bitcast(f32r), pt[:], COPY)
                    else:
                        nc.scalar.activation(dst, pt[:], COPY)

        def transpose_rows(src_d, row0, rows, dst, sb, ps, dt_out):
            """DMA rows [row0, row0+rows) of src_d, transpose into
            dst [P, DC, rows]."""
            for rt in range(rows // P):
                nat = sb.tile([P, D], f32, tag="wnat")
                nc.sync.dma_start(
                    nat[:], src_d[row0 + rt * P:row0 + (rt + 1) * P, :])
                if dt_out is f16:
                    c16 = sb.tile([P, D], f16, tag="nat16")
                    nc.vector.tensor_copy(c16[:], nat[:])
                    for dt in range(DC):
                        pt = ps.tile([P, P], f16, tag="tp16")
                        nc.tensor.transpose(pt[:], c16[:, dt * P:(dt + 1) * P],
                                            ident16[:])
                        nc.vector.tensor_copy(dst[:, dt, rt * P:(rt + 1) * P],
                                              pt[:])
                else:
                    for dt in range(DC):
                        pt = ps.tile([P, P], f32, tag="tpw")
                        nc.tensor.transpose(pt[:], nat[:, dt * P:(dt + 1) * P],
                                            ident32[:])
                        nc.vector.tensor_copy(
                            dst[:, dt, rt * P:(rt + 1) * P].bitcast(f32r), pt[:])

        def project(wT, xT, out_cb, ps2, nmm=None):
            """out[jt] = (wT.T @ xT) per jt chunk; out_cb(jt, psum)."""
            width = xT.shape[2] if nmm is None else nmm
            for jt in range(DC):
                pj = ps2.tile([P, width], f32, tag="pq", name=f"pj_{jt}")
                for dt in range(DC):
                    nc.tensor.matmul(pj[:],
                                     wT[:, dt, jt * P:(jt + 1) * P].bitcast(f32r),
                                     xT[:, dt, :].bitcast(f32r),
                                     start=(dt == 0), stop=(dt == DC - 1))
                out_cb(jt, pj)

        def scores_panel(col0, kpSrc, b_ps3):
            """scores[:, :, col0:col0+width] for all IT row-tiles."""
            width = kpSrc.shape[2]
            for it in range(IT):
                psc = b_ps3.tile([P, width], f32, tag="ps", name=f"psc_{it}")
                for jt in range(DC):
                    nc.tensor.matmul(
                        psc[:],
                        qpT[:, jt, it * P:(it + 1) * P].bitcast(f32r),
                        kpSrc[:, jt, :].bitcast(f32r),
                        start=(jt == 0), stop=(jt == DC - 1))
                nc.vector.tensor_copy(
                    scoresR[:, it, col0:col0 + width], psc[:])

        # ---- phase A: kpT_own -> AllGather; qpT; local head panels ---
        poolA_cm = tc.tile_pool(name="qpT", bufs=1)   # qpT: A..B1 end
        poolA = poolA_cm.__enter__()
        qpT = poolA.tile([P, DC, B], f32, tag="qpT")

        a_w_cm = tc.tile_pool(name="a_w", bufs=1)     # wkT: A..B1a end
        a_w = a_w_cm.__enter__()
        wkT = a_w.tile([P, DC, D], f32, tag="wkT")

        with (
            tc.tile_pool(name="a_q", bufs=1) as a_q,
            tc.tile_pool(name="a_sb", bufs=3) as a_sb,
            tc.tile_pool(name="a_ps", bufs=3, space="PSUM") as a_ps,
            tc.tile_pool(name="a_ps2", bufs=2, space="PSUM") as a_ps2,
        ):
            # --- k path first so the AllGather launches early ---
            transpose_weight("wk", wkT, a_sb, a_ps, f32r)
            kTo = a_q.tile([P, DC, B], f32, tag="kTo")
            transpose_rows(k_d, 0, B, kTo, a_sb, a_ps, f32r)
            kpTo = a_q.tile([P, DC, B], f32, tag="kpTo")

            def kp_cb(jt, pj):
                nc.scalar.activation(kpTo[:, jt, :].bitcast(f32r), pj[:], IDENT,
                                     bias=kb_col[:, jt:jt + 1])
                nc.sync.dma_start(kp_in[jt * P:(jt + 1) * P, :], kpTo[:, jt, :])
                if (jt + 1) * P == JH or (jt + 1) * P == D:
                    j = 0 if (jt + 1) * P == JH else 1
                    nc.gpsimd.collective_compute(
                        "AllGather", mybir.AluOpType.bypass,
                        replica_groups=[list(range(N_CORES))],
                        ins=[kp_in[j * JH:(j + 1) * JH, :].opt()],
                        outs=[kp_all_j[j][:].opt()])
            project(wkT, kTo, kp_cb, a_ps2)

            # --- q path (overlaps the gather) ---
            wqT = a_q.tile([P, DC, D], f32, tag="wqT")
            transpose_weight("wq", wqT, a_sb, a_ps, f32r)
            qT = a_q.tile([P, DC, B], f32, tag="qT")
            transpose_rows(q_d, 0, B, qT, a_sb, a_ps, f32r)

            def qp_cb(jt, pj):
                nc.scalar.activation(qpT[:, jt, :].bitcast(f32r), pj[:], IDENT,
                                     bias=qb_col[:, jt:jt + 1])
            project(wqT, qT, qp_cb, a_ps2)

        poolC_cm = tc.tile_pool(name="woT", bufs=1, side="right")   # woT: ..D
        poolC = poolC_cm.__enter__()
        woT = poolC.tile([P, DC, D], f16, tag="woT")

        # ---- B1a: local head panels (fill the AllGather window) ------
        poolB_cm = tc.tile_pool(name="scores", bufs=1, side="right")
        poolB = poolB_cm.__enter__()
        scoresR = poolB.tile([P, IT, N], f32, tag="scores")

        if n_local:
            with (
                tc.tile_pool(name="l_sb", bufs=1) as l_sb,
                tc.tile_pool(name="l_k", bufs=3) as l_k,
                tc.tile_pool(name="l_ps", bufs=2, space="PSUM") as l_ps,
                tc.tile_pool(name="l_ps2", bufs=2, space="PSUM") as l_ps2,
                tc.tile_pool(name="l_ps3", bufs=4, space="PSUM") as l_ps3,
            ):
                for lp in range(n_local):
                    kTl = l_sb.tile([P, DC, B], f32, tag="kTl")
                    transpose_rows(kh_d, lp * B, B, kTl, l_k, l_ps, f32r)
                    kpTl = l_sb.tile([P, DC, B], f32, tag="kpTl")

                    def kpl_cb(jt, pj, kpTl=kpTl):
                        nc.scalar.activation(kpTl[:, jt, :].bitcast(f32r),
                                             pj[:], IDENT,
                                             bias=kb_col[:, jt:jt + 1])
                    project(wkT, kTl, kpl_cb, l_ps2)
                    scores_panel(lp * PW, kpTl, l_ps3)
        a_w_cm.__exit__(None, None, None)   # free wkT

        # ---- phase C: vp_own -> AllGather; woT -----------------------
        with (
            tc.tile_pool(name="c_w", bufs=1) as c_w,
            tc.tile_pool(name="c_sb", bufs=3) as c_sb,
            tc.tile_pool(name="c_ps", bufs=3, space="PSUM") as c_ps,
            tc.tile_pool(name="c_ps2", bufs=2, space="PSUM") as c_ps2,
        ):
            wvT = c_w.tile([P, DC, D], f16, tag="wvT")
            transpose_weight("wv", wvT, c_sb, c_ps, f16)
            vTo = c_w.tile([P, DC, B], f16, tag="vTo")
            transpose_rows(v_d, 0, B, vTo, c_sb, c_ps, f16)
            vpo = c_w.tile([P, IT, D], f16, tag="vpo")
            for lc in range(IT):
                for ms in range(MS):
                    pv = c_ps2.tile([P, FS], f32, tag="pv")
                    for dt in range(DC):
                        nc.tensor.matmul(pv[:], vTo[:, dt, lc * P:(lc + 1) * P],
                                         wvT[:, dt, ms * FS:(ms + 1) * FS],
                                         start=(dt == 0), stop=(dt == DC - 1))
                    nc.vector.tensor_add(vpo[:, lc, ms * FS:(ms + 1) * FS],
                                         pv[:], vb_bc[:, ms * FS:(ms + 1) * FS])
                nc.sync.dma_start(vp_in[lc * P:(lc + 1) * P, :], vpo[:, lc, :])
            nc.gpsimd.collective_compute(
                "AllGather", mybir.AluOpType.bypass,
                replica_groups=[list(range(N_CORES))],
                ins=[vp_in[:].opt()], outs=[vp_all[:].opt()])

            # wo transposes fill the gather wait
            transpose_weight("wo", woT, c_sb, c_ps, f16)

        # ---- phase B1b: gathered kpT panels -> scores ----------------
        JC = JH // P            # jt chunks per feature-half
        with (
            tc.tile_pool(name="b_sb", bufs=2) as b_sb,
            tc.tile_pool(name="b_ps3", bufs=1, space="PSUM") as b_ps3,
        ):
            panels = list(range(n_local, LPAN))
            for w in range(0, len(panels), 2):
                wave = panels[w:w + 2]
                kpPs, pscs = {}, {}
                # pass 1: feature-half 0 for both panels in the wave
                for wi, lp in enumerate(wave):
                    kpP = b_sb.tile([P, DC, PW], f32, tag=f"kpP{wi}",
                                    name=f"kpP{wi}")
                    kpPs[lp] = kpP
                    for jt in range(JC):
                        nc.sync.dma_start(
                            kpP[:, jt, :].bitcast(f32r),
                            kp_all_j[0][lp, jt * P:(jt + 1) * P, :].bitcast(f32r))
                    for it in range(IT):
                        psc = b_ps3.tile([P, PW], f32, tag=f"ps{wi}_{it}",
                                         name=f"psc{wi}_{it}")
                        pscs[(lp, it)] = psc
                        for jt in range(JC):
                            nc.tensor.matmul(
                                psc[:],
                                qpT[:, jt, it * P:(it + 1) * P].bitcast(f32r),
                                kpP[:, jt, :].bitcast(f32r),
                                start=(jt == 0), stop=False)
                # pass 2: feature-half 1, finish accumulation + copy back
                for lp in wave:
                    kpP = kpPs[lp]
                    for jt in range(JC, DC):
                        nc.scalar.dma_start(
                            kpP[:, jt, :].bitcast(f32r),
                            kp_all_j[1][lp, (jt - JC) * P:(jt - JC + 1) * P, :]
                            .bitcast(f32r))
                    for it in range(IT):
                        psc = pscs[(lp, it)]
                        for jt in range(JC, DC):
                            nc.tensor.matmul(
                                psc[:],
                                qpT[:, jt, it * P:(it + 1) * P].bitcast(f32r),
                                kpP[:, jt, :].bitcast(f32r),
                                start=False, stop=(jt == DC - 1))
                        nc.vector.tensor_copy(
                            scoresR[:, it, lp * PW:(lp + 1) * PW], psc[:])
        poolA_cm.__exit__(None, None, None)   # free qpT

        # ---- merged B2 + D: softmax, attnT, x1, out ------------------
        poolBD_cm = tc.tile_pool(name="attnT", bufs=1)
        poolBD = poolBD_cm.__enter__()
        attnT = poolBD.tile([P, LT, B], f16, tag="attnT")
        vp = poolBD.tile([P, LT, D], f16, tag="vp")

        # vp: DMA gathered [N_CORES, B, D] -> [P, LT, D]
        for lc in range(LT):
            c, r = divmod(lc, IT)
            nc.gpsimd.dma_start(vp[:, lc, :], vp_all[c, r * P:(r + 1) * P, :])

        with (
            tc.tile_pool(name="s_sb", bufs=2) as s_sb,
            tc.tile_pool(name="d_sb", bufs=1) as d_sb,
            tc.tile_pool(name="s_ps", bufs=4, space="PSUM") as s_ps,
            tc.tile_pool(name="d_ps", bufs=1, space="PSUM") as d_ps,
            tc.tile_pool(name="d_ps2", bufs=1, space="PSUM") as d_ps2,
        ):
            for it in range(IT):
                sc = scoresR[:, it, :]
                nmax = s_sb.tile([P, 1], f32, tag="nmax")
                nc.vector.reduce_max(nmax[:], sc, axis=mybir.AxisListType.X,
                                     negate=True)
                zsum = s_sb.tile([P, 1], f32, tag="zsum")
                nc.scalar.activation(sc, sc, EXP, bias=nmax[:], scale=1.0,
                                     accum_out=zsum[:])
                zinv = s_sb.tile([P, 1], f32, tag="zinv")
                nc.vector.reciprocal(zinv[:], zsum[:])
                # ACT fuses normalize+fp16 cast (scale=1/Z) feeding the
                # transposes; the DVE f32 normalize for the attn output runs
                # in parallel off the critical path.
                for lg in range(N // FS):
                    ab = s_sb.tile([P, FS], f16, tag="abf")
                    nc.scalar.activation(ab[:], sc[:, lg * FS:(lg + 1) * FS],
                                         COPY, scale=zinv[:])
                    for li in range(FS // P):
                        lt = lg * (FS // P) + li
                        pt = s_ps.tile([P, P], f16, tag="tp16")
                        nc.tensor.transpose(pt[:], ab[:, li * P:(li + 1) * P],
                                            ident16[:])
                        dst = attnT[:, lt, it * P:(it + 1) * P]
                        if lt % 2 == 0:
                            nc.vector.tensor_copy(dst, pt[:])
                        else:
                            nc.scalar.activation(dst, pt[:], COPY)
                nc.vector.tensor_scalar_mul(sc, sc, zinv[:])
                nc.sync.dma_start(a_out[it * P:(it + 1) * P, :], sc)

                x1b = d_sb.tile([P, D], f16, tag="x1b")
                pxs = [d_ps.tile([P, FS], f32, tag=f"px{ms}", name=f"px{ms}")
                       for ms in range(MS)]
                for lt in range(LT):
                    for ms in range(MS):
                        nc.tensor.matmul(pxs[ms][:],
                                         attnT[:, lt, it * P:(it + 1) * P],
                                         vp[:, lt, ms * FS:(ms + 1) * FS],
                                         start=(lt == 0), stop=(lt == LT - 1))
                for ms in range(MS):
                    nc.scalar.activation(x1b[:, ms * FS:(ms + 1) * FS],
                                         pxs[ms][:], COPY)
                x1T = d_sb.tile([P, DC, P], f16, tag="x1T")
                for mt in range(DC):
                    pt = s_ps.tile([P, P], f16, tag="tp16")
                    nc.tensor.transpose(pt[:], x1b[:, mt * P:(mt + 1) * P],
                                        ident16[:])
                    nc.vector.tensor_copy(x1T[:, mt, :], pt[:])
                xo = d_sb.tile([P, D], f32, tag="xo")
                pxos = [d_ps2.tile([P, FS], f32, tag=f"pxo{cs}", name=f"pxo{cs}")
                        for cs in range(MS)]
                for mt in range(DC):
                    for cs in range(MS):
                        nc.tensor.matmul(pxos[cs][:], x1T[:, mt, :],
                                         woT[:, mt, cs * FS:(cs + 1) * FS],
                                         start=(mt == 0), stop=(mt == DC - 1))
                for cs in range(MS):
                    nc.vector.tensor_add(xo[:, cs * FS:(cs + 1) * FS],
                                         pxos[cs][:],
                                         ob_bc[:, cs * FS:(cs + 1) * FS])
                nc.scalar.dma_start(x_out[it * P:(it + 1) * P, :], xo[:])
        poolB_cm.__exit__(None, None, None)   # free scoresR

        poolBD_cm.__exit__(None, None, None)
        poolC_cm.__exit__(None, None, None)
        dram_cm.__exit__(None, None, None)
        const_cm.__exit__(None, None, None)

    nc.compile()
    return nc


_built = {}


def _get_nc(N=4096, D=1024):
    key = (N, D)
    if key not in _built:
        _built[key] = build(N, D)
    return _built[key]


def _make_in_maps(inputs):
    q = np.ascontiguousarray(np.asarray(inputs["q"], dtype=np.float32))
    k = np.ascontiguousarray(np.asarray(inputs["k"], dtype=np.float32))
    v = np.ascontiguousarray(np.asarray(inputs["v"], dtype=np.float32))
    N, D = k.shape
    B = N // N_CORES
    shared = {}
    for nm in ("wq", "wk", "wv", "wo"):
        shared[nm + "_w"] = np.ascontiguousarray(
            np.asarray(inputs[nm + "_w"], dtype=np.float32))
        shared[nm + "_b"] = np.ascontiguousarray(
            np.asarray(inputs[nm + "_b"], dtype=np.float32))
    if N_LOCAL:
        shared["k_head"] = np.ascontiguousarray(k[:N_LOCAL * B])
    return [dict(shared,
                 q=q[c * B:(c + 1) * B],
                 k=np.ascontiguousarray(k[c * B:(c + 1) * B]),
                 v=np.ascontiguousarray(v[c * B:(c + 1) * B]))
            for c in range(N_CORES)], N, D


def kernel(**inputs):
    in_maps, N, D = _make_in_maps(inputs)
    nc = _get_nc(N, D)
    res = run_bass_kernel_spmd(nc, in_maps, core_ids=list(range(N_CORES)))
    x = np.concatenate([res.results[c]["x_out"] for c in range(N_CORES)], axis=0)
    attn = np.concatenate([res.results[c]["attn_out"] for c in range(N_CORES)],
                          axis=0)
    return (x, attn)


# revision 39
# speedup vs baseline: 1.0270x; 1.0270x over previous
"""Multi-head-attention (single-head, no scaling) Bass kernel for 8 trn2 cores.

v2: distributed K/V projections + AllGather.

Sharding: core c owns q rows [c*B, (c+1)*B) AND k/v rows [c*B, (c+1)*B).
Weights replicated. Each core:
  A)  wk transpose (PE, f32 -> rounded f32r copyback), own-k transpose,
      kpT_own = (wk @ k_own.T + b) [D, B] f32r -> DRAM -> AllGather (CC queue)
      wq transpose, qT, qpT (overlaps the AllGather)
  C)  wv (fp16) transpose, own-v cast+transpose, vp_own [B, D] fp16
      (+bias) -> DRAM -> AllGather; wo (fp16) transpose
  B1) for each 512-panel: DMA gathered kpT panel, DVE re-round to f32r,
      scores matmuls -> resident [B, N] f32
  B2) per 128-row tile: softmax (DVE/ACT, fused exp+rowsum), DMA attn out,
      cast fp16 + PE transpose -> attnT
  Dv) DMA gathered vp -> SBUF fp16
  D)  x1 = attn @ vp, transpose, x = x1 @ wo.T + b -> DMA out
"""

import numpy as np

_DIRECT_F32R = True

try:
    import concourse.bass  # noqa: F401
except ImportError:
    import sys
    for _p in ("/opt/trn_rl_repo", "/root/.axon_site/_ro/trn_rl_repo"):
        if _p not in sys.path:
            sys.path.append(_p)

import concourse.bacc as bacc
import concourse.mybir as mybir
from concourse.bass_utils import run_bass_kernel_spmd
from concourse.tile import TileContext
from concourse.masks import make_identity

P = 128
N_CORES = 8
N_LOCAL = 0   # head panels computed locally on every core

f32 = mybir.dt.float32
f32r = mybir.dt.float32r
f16 = mybir.dt.float16
EXP = mybir.ActivationFunctionType.Exp
COPY = mybir.ActivationFunctionType.Copy
IDENT = mybir.ActivationFunctionType.Identity


def build(N=4096, D=1024, n_local=N_LOCAL):
    B = N // N_CORES            # q/k/v rows per core
    DC = D // P                 # contraction chunks (d)
    IT = B // P                 # 128-row tiles per core slice
    PW = B                      # scores panel width == per-core slice
    LPAN = N // PW
    LT = N // P
    FS = min(512, D)
    MS = D // FS
    LHEAD = n_local * B         # head rows computed locally on every core

    nc = bacc.Bacc("TRN2", target_bir_lowering=False, debug=False,
                   num_devices=N_CORES)

    q_d = nc.dram_tensor("q", [B, D], f32, kind="ExternalInput")
    k_d = nc.dram_tensor("k", [B, D], f32, kind="ExternalInput")
    v_d = nc.dram_tensor("v", [B, D], f32, kind="ExternalInput")
    if n_local:
        kh_d = nc.dram_tensor("k_head", [LHEAD, D], f32, kind="ExternalInput")
    w_d = {}
    b_d = {}
    for nm in ("wq", "wk", "wv", "wo"):
        w_d[nm] = nc.dram_tensor(nm + "_w", [D, D], f32, kind="ExternalInput")
        b_d[nm] = nc.dram_tensor(nm + "_b", [D], f32, kind="ExternalInput")
    x_out = nc.dram_tensor("x_out", [B, D], f32, kind="ExternalOutput")
    a_out = nc.dram_tensor("attn_out", [B, N], f32, kind="ExternalOutput")

    with TileContext(nc) as tc:
        const_cm = tc.tile_pool(name="const", bufs=1)
        const = const_cm.__enter__()
        dram_cm = tc.tile_pool(name="dram", bufs=1, space="DRAM")
        dram = dram_cm.__enter__()

        kp_in = dram.tile([D, B], f32, tag="kp_in")
        JH = D // 2             # feature-half split of the kp gather
        kp_all_j = [dram.tile([N_CORES, JH, B], f32, tag=f"kp_all{j}",
                              name=f"kp_all{j}") for j in range(2)]
        vp_in = dram.tile([B, D], f16, tag="vp_in")
        vp_all = dram.tile([N_CORES, B, D], f16, tag="vp_all")
        warm_in = dram.tile([1, 32], f32, tag="warm_in")
        warm_out = dram.tile([N_CORES, 32], f32, tag="warm_out")

        # tiny warm-up collective: absorbs comm-init/rendezvous cost while
        # the input DMAs and weight transposes run
        warm_sb = const.tile([1, 32], f32, tag="warm")
        nc.gpsimd.memset(warm_sb[:], 0.0)
        nc.gpsimd.dma_start(warm_in[:], warm_sb[:])
        nc.gpsimd.collective_compute(
            "AllGather", mybir.AluOpType.bypass,
            replica_groups=[list(range(N_CORES))],
            ins=[warm_in[:].opt()], outs=[warm_out[:].opt()])

        ident32 = const.tile([P, P], f32, tag="id32")
        make_identity(nc, ident32[:])
        ident16 = const.tile([P, P], f16, tag="id16")
        make_identity(nc, ident16[:])

        qb_col = const.tile([P, DC], f32, tag="qbcol")
        kb_col = const.tile([P, DC], f32, tag="kbcol")
        for jt in range(DC):
            nc.sync.dma_start(qb_col[:, jt:jt + 1],
                              b_d["wq"][jt * P:(jt + 1) * P])
            nc.sync.dma_start(kb_col[:, jt:jt + 1],
                              b_d["wk"][jt * P:(jt + 1) * P])
        vb_bc = const.tile([P, D], f16, tag="vbbc")
        ob_bc = const.tile([P, D], f16, tag="obbc")
        with tc.tile_pool(name="btmp", bufs=2) as btmp:
            for bc_t, bnm in ((vb_bc, "wv"), (ob_bc, "wo")):
                tmp = btmp.tile([P, D], f32, tag="btmp")
                nc.sync.dma_start(
                    tmp[:],
                    b_d[bnm].rearrange("(o d) -> o d", o=1).partition_broadcast(P))
                nc.vector.tensor_copy(bc_t[:], tmp[:])

        def transpose_weight(nm, wT, sb, ps, rnd):
            """DMA weight natural, PE-transpose 128x128 blocks into wT."""
            for jt in range(DC):
                wnat = sb.tile([P, D], f32, tag="wnat")
                nc.sync.dma_start(wnat[:], w_d[nm][jt * P:(jt + 1) * P, :])
                for dt in range(DC):
                    pt = ps.tile([P, P], f32, tag="tpw")
                    nc.tensor.transpose(pt[:], wnat[:, dt * P:(dt + 1) * P],
                                        ident32[:])
                    dst = wT[:, dt, jt * P:(jt + 1) * P]
                    if rnd is f32r:
                        dst = dst.bitcast(f32r)
                    # balance copybacks across ACT and DVE
                    if dt % 2 == 0:
                        nc.scalar.activation(dst, pt[:], COPY)
                    else:
                        nc.vector.tensor_copy(dst, pt[:])

        def transpose_rows(src_d, row0, rows, dst, sb, ps, dt_out):
            """DMA rows [row0, row0+rows) of src_d, transpose into
            dst [P, DC, rows]."""
            for rt in range(rows // P):
                nat = sb.tile([P, D], f32, tag="wnat")
                nc.sync.dma_start(
                    nat[:], src_d[row0 + rt * P:row0 + (rt + 1) * P, :])
                if dt_out is f16:
                    c16 = sb.tile([P, D], f16, tag="nat16")
                    nc.vector.tensor_copy(c16[:], nat[:])
                    for dt in range(DC):
                        pt = ps.tile([P, P], f16, tag="tp16")
                        nc.tensor.transpose(pt[:], c16[:, dt * P:(dt + 1) * P],
                                            ident16[:])
                        nc.vector.tensor_copy(dst[:, dt, rt * P:(rt + 1) * P],
                                              pt[:])
                else:
                    for dt in range(DC):
                        pt = ps.tile([P, P], f32, tag="tpw")
                        nc.tensor.transpose(pt[:], nat[:, dt * P:(dt + 1) * P],
                                            ident32[:])
                        nc.vector.tensor_copy(
                            dst[:, dt, rt * P:(rt + 1) * P].bitcast(f32r), pt[:])

        def project(wT, xT, out_cb, ps2, nmm=None):
            """out[jt] = (wT.T @ xT) per jt chunk; out_cb(jt, psum)."""
            width = xT.shape[2] if nmm is None else nmm
            for jt in range(DC):
                pj = ps2.tile([P, width], f32, tag="pq", name=f"pj_{jt}")
                for dt in range(DC):
                    nc.tensor.matmul(pj[:],
                                     wT[:, dt, jt * P:(jt + 1) * P].bitcast(f32r),
                                     xT[:, dt, :].bitcast(f32r),
                                     start=(dt == 0), stop=(dt == DC - 1))
                out_cb(jt, pj)

        def scores_panel(col0, kpSrc, b_ps3):
            """scores[:, :, col0:col0+width] for all IT row-tiles."""
            width = kpSrc.shape[2]
            for it in range(IT):
                psc = b_ps3.tile([P, width], f32, tag="ps", name=f"psc_{it}")
                for jt in range(DC):
                    nc.tensor.matmul(
                        psc[:],
                        qpT[:, jt, it * P:(it + 1) * P].bitcast(f32r),
                        kpSrc[:, jt, :].bitcast(f32r),
                        start=(jt == 0), stop=(jt == DC - 1))
                nc.vector.tensor_copy(
                    scoresR[:, it, col0:col0 + width], psc[:])

        # ---- phase A: kpT_own -> AllGather; qpT; local head panels ---
        poolA_cm = tc.tile_pool(name="qpT", bufs=1)   # qpT: A..B1 end
        poolA = poolA_cm.__enter__()
        qpT = poolA.tile([P, DC, B], f32, tag="qpT")

        a_w_cm = tc.tile_pool(name="a_w", bufs=1)     # wkT: A..B1a end
        a_w = a_w_cm.__enter__()
        wkT = a_w.tile([P, DC, D], f32, tag="wkT")

        with (
            tc.tile_pool(name="a_q", bufs=1) as a_q,
            tc.tile_pool(name="a_sb", bufs=3) as a_sb,
            tc.tile_pool(name="a_ps", bufs=3, space="PSUM") as a_ps,
            tc.tile_pool(name="a_ps2", bufs=2, space="PSUM") as a_ps2,
        ):
            # --- k path first so the AllGather launches early ---
            transpose_weight("wk", wkT, a_sb, a_ps, f32r)
            kTo = a_q.tile([P, DC, B], f32, tag="kTo")
            transpose_rows(k_d, 0, B, kTo, a_sb, a_ps, f32r)
            kpTo = a_q.tile([P, DC, B], f32, tag="kpTo")

            def kp_cb(jt, pj):
                nc.scalar.activation(kpTo[:, jt, :].bitcast(f32r), pj[:], IDENT,
                                     bias=kb_col[:, jt:jt + 1])
                nc.sync.dma_start(kp_in[jt * P:(jt + 1) * P, :], kpTo[:, jt, :])
                if (jt + 1) * P == JH or (jt + 1) * P == D:
                    j = 0 if (jt + 1) * P == JH else 1
                    nc.gpsimd.collective_compute(
                        "AllGather", mybir.AluOpType.bypass,
                        replica_groups=[list(range(N_CORES))],
                        ins=[kp_in[j * JH:(j + 1) * JH, :].opt()],
                        outs=[kp_all_j[j][:].opt()])
            project(wkT, kTo, kp_cb, a_ps2)

            # --- q path (overlaps the gather) ---
            wqT = a_q.tile([P, DC, D], f32, tag="wqT")
            transpose_weight("wq", wqT, a_sb, a_ps, f32r)
            qT = a_q.tile([P, DC, B], f32, tag="qT")
            transpose_rows(q_d, 0, B, qT, a_sb, a_ps, f32r)

            def qp_cb(jt, pj):
                nc.scalar.activation(qpT[:, jt, :].bitcast(f32r), pj[:], IDENT,
                                     bias=qb_col[:, jt:jt + 1])
            project(wqT, qT, qp_cb, a_ps2)

        poolC_cm = tc.tile_pool(name="woT", bufs=1, side="right")   # woT: ..D
        poolC = poolC_cm.__enter__()
        woT = poolC.tile([P, DC, D], f16, tag="woT")

        # ---- B1a: local head panels (fill the AllGather window) ------
        poolB_cm = tc.tile_pool(name="scores", bufs=1, side="right")
        poolB = poolB_cm.__enter__()
        scoresR = poolB.tile([P, IT, N], f32, tag="scores")

        if n_local:
            with (
                tc.tile_pool(name="l_sb", bufs=1) as l_sb,
                tc.tile_pool(name="l_k", bufs=3) as l_k,
                tc.tile_pool(name="l_ps", bufs=2, space="PSUM") as l_ps,
                tc.tile_pool(name="l_ps2", bufs=2, space="PSUM") as l_ps2,
                tc.tile_pool(name="l_ps3", bufs=4, space="PSUM") as l_ps3,
            ):
                for lp in range(n_local):
                    kTl = l_sb.tile([P, DC, B], f32, tag="kTl")
                    transpose_rows(kh_d, lp * B, B, kTl, l_k, l_ps, f32r)
                    kpTl = l_sb.tile([P, DC, B], f32, tag="kpTl")

                    def kpl_cb(jt, pj, kpTl=kpTl):
                        nc.scalar.activation(kpTl[:, jt, :].bitcast(f32r),
                                             pj[:], IDENT,
                                             bias=kb_col[:, jt:jt + 1])
                    project(wkT, kTl, kpl_cb, l_ps2)
                    scores_panel(lp * PW, kpTl, l_ps3)
        a_w_cm.__exit__(None, None, None)   # free wkT

        # ---- phase C: vp_own -> AllGather; woT -----------------------
        with (
            tc.tile_pool(name="c_w", bufs=1) as c_w,
            tc.tile_pool(name="c_sb", bufs=3) as c_sb,
            tc.tile_pool(name="c_ps", bufs=3, space="PSUM") as c_ps,
            tc.tile_pool(name="c_ps2", bufs=2, space="PSUM") as c_ps2,
        ):
            wvT = c_w.tile([P, DC, D], f16, tag="wvT")
            transpose_weight("wv", wvT, c_sb, c_ps, f16)
            vTo = c_w.tile([P, DC, B], f16, tag="vTo")
            transpose_rows(v_d, 0, B, vTo, c_sb, c_ps, f16)
            vpo = c_w.tile([P, IT, D], f16, tag="vpo")
            for lc in range(IT):
                for ms in range(MS):
                    pv = c_ps2.tile([P, FS], f32, tag="pv")
                    for dt in range(DC):
                        nc.tensor.matmul(pv[:], vTo[:, dt, lc * P:(lc + 1) * P],
                                         wvT[:, dt, ms * FS:(ms + 1) * FS],
                                         start=(dt == 0), stop=(dt == DC - 1))
                    nc.vector.tensor_add(vpo[:, lc, ms * FS:(ms + 1) * FS],
                                         pv[:], vb_bc[:, ms * FS:(ms + 1) * FS])
                nc.sync.dma_start(vp_in[lc * P:(lc + 1) * P, :], vpo[:, lc, :])
            nc.gpsimd.collective_compute(
                "AllGather", mybir.AluOpType.bypass,
                replica_groups=[list(range(N_CORES))],
                ins=[vp_in[:].opt()], outs=[vp_all[:].opt()])

            # wo transposes fill the gather wait
            transpose_weight("wo", woT, c_sb, c_ps, f16)

        # ---- phase B1b: gathered kpT panels -> scores ----------------
        JC = JH // P            # jt chunks per feature-half
        with (
            tc.tile_pool(name="b_sb", bufs=2) as b_sb,
            tc.tile_pool(name="b_ps3", bufs=1, space="PSUM") as b_ps3,
        ):
            panels = list(range(n_local, LPAN))
            for w in range(0, len(panels), 2):
                wave = panels[w:w + 2]
                kpPs, pscs = {}, {}
                # pass 1: feature-half 0 for both panels in the wave
                for wi, lp in enumerate(wave):
                    kpP = b_sb.tile([P, DC, PW], f32, tag=f"kpP{wi}",
                                    name=f"kpP{wi}")
                    kpPs[lp] = kpP
                    for jt in range(JC):
                        nc.sync.dma_start(
                            kpP[:, jt, :].bitcast(f32r),
                            kp_all_j[0][lp, jt * P:(jt + 1) * P, :].bitcast(f32r))
                    for it in range(IT):
                        psc = b_ps3.tile([P, PW], f32, tag=f"ps{wi}_{it}",
                                         name=f"psc{wi}_{it}")
                        pscs[(lp, it)] = psc
                        for jt in range(JC):
                            nc.tensor.matmul(
                                psc[:],
                                qpT[:, jt, it * P:(it + 1) * P].bitcast(f32r),
                                kpP[:, jt, :].bitcast(f32r),
                                start=(jt == 0), stop=False)
                # pass 2: feature-half 1, finish accumulation + copy back
                for lp in wave:
                    kpP = kpPs[lp]
                    for jt in range(JC, DC):
                        nc.scalar.dma_start(
                            kpP[:, jt, :].bitcast(f32r),
                            kp_all_j[1][lp, (jt - JC) * P:(jt - JC + 1) * P, :]
                            .bitcast(f32r))
                    for it in range(IT):
                        psc = pscs[(lp, it)]
                        for jt in range(JC, DC):
                            nc.tensor.matmul(
                                psc[:],
                                qpT[:, jt, it * P:(it + 1) * P].bitcast(f32r),
                                kpP[:, jt, :].bitcast(f32r),
                                start=False, stop=(jt == DC - 1))
                        nc.vector.tensor_copy(
                            scoresR[:, it, lp * PW:(lp + 1) * PW], psc[:])
        poolA_cm.__exit__(None, None, None)   # free qpT

        # ---- merged B2 + D: softmax, attnT, x1, out ------------------
        poolBD_cm = tc.tile_pool(name="attnT", bufs=1)
        poolBD = poolBD_cm.__enter__()
        attnT = poolBD.tile([P, LT, B], f16, tag="attnT")
        vp = poolBD.tile([P, LT, D], f16, tag="vp")

        # vp: DMA gathered [N_CORES, B, D] -> [P, LT, D]
        for lc in range(LT):
            c, r = divmod(lc, IT)
            nc.gpsimd.dma_start(vp[:, lc, :], vp_all[c, r * P:(r + 1) * P, :])

        with (
            tc.tile_pool(name="s_sb", bufs=2) as s_sb,
            tc.tile_pool(name="d_sb", bufs=1) as d_sb,
            tc.tile_pool(name="s_ps", bufs=4, space="PSUM") as s_ps,
            tc.tile_pool(name="d_ps", bufs=2, space="PSUM") as d_ps,
            tc.tile_pool(name="d_ps2", bufs=2, space="PSUM") as d_ps2,
        ):
            for it in range(IT):
                sc = scoresR[:, it, :]
                nmax = s_sb.tile([P, 1], f32, tag="nmax")
                nc.vector.reduce_max(nmax[:], sc, axis=mybir.AxisListType.X,
                                     negate=True)
                zsum = s_sb.tile([P, 1], f32, tag="zsum")
                nc.scalar.activation(sc, sc, EXP, bias=nmax[:], scale=1.0,
                                     accum_out=zsum[:])
                zinv = s_sb.tile([P, 1], f32, tag="zinv")
                nc.vector.reciprocal(zinv[:], zsum[:])
                # ACT fuses normalize+fp16 cast (scale=1/Z) feeding the
                # transposes; the DVE f32 normalize for the attn output runs
                # in parallel off the critical path.
                for lg in range(N // FS):
                    ab = s_sb.tile([P, FS], f16, tag="abf")
                    nc.scalar.activation(ab[:], sc[:, lg * FS:(lg + 1) * FS],
                                         COPY, scale=zinv[:])
                    for li in range(FS // P):
                        lt = lg * (FS // P) + li
                        pt = s_ps.tile([P, P], f16, tag="tp16")
                        nc.tensor.transpose(pt[:], ab[:, li * P:(li + 1) * P],
                                            ident16[:])
                        dst = attnT[:, lt, it * P:(it + 1) * P]
                        if lt % 2 == 0:
                            nc.vector.tensor_copy(dst, pt[:])
                        else:
                            nc.scalar.activation(dst, pt[:], COPY)
                nc.vector.tensor_scalar_mul(sc, sc, zinv[:])
                nc.sync.dma_start(a_out[it * P:(it + 1) * P, :], sc)

                x1b = d_sb.tile([P, D], f16, tag="x1b")
                for ms in range(MS):
                    px = d_ps.tile([P, FS], f32, tag="px")
                    for lt in range(LT):
                        nc.tensor.matmul(px[:], attnT[:, lt, it * P:(it + 1) * P],
                                         vp[:, lt, ms * FS:(ms + 1) * FS],
                                         start=(lt == 0), stop=(lt == LT - 1))
                    nc.scalar.activation(x1b[:, ms * FS:(ms + 1) * FS], px[:],
                                         COPY)
                x1T = d_sb.tile([P, DC, P], f16, tag="x1T")
                for mt in range(DC):
                    pt = s_ps.tile([P, P], f16, tag="tp16")
                    nc.tensor.transpose(pt[:], x1b[:, mt * P:(mt + 1) * P],
                                        ident16[:])
                    nc.vector.tensor_copy(x1T[:, mt, :], pt[:])
                xo = d_sb.tile([P, D], f32, tag="xo")
                for cs in range(MS):
                    pxo = d_ps2.tile([P, FS], f32, tag="pxo")
                    for mt in range(DC):
                        nc.tensor.matmul(pxo[:], x1T[:, mt, :],
                                         woT[:, mt, cs * FS:(cs + 1) * FS],
                                         start=(mt == 0), stop=(mt == DC - 1))
                    nc.vector.tensor_add(xo[:, cs * FS:(cs + 1) * FS], pxo[:],
                                         ob_bc[:, cs * FS:(cs + 1) * FS])
                nc.scalar.dma_start(x_out[it * P:(it + 1) * P, :], xo[:])
        poolB_cm.__exit__(None, None, None)   # free scoresR

        poolBD_cm.__exit__(None, None, None)
        poolC_cm.__exit__(None, None, None)
        dram_cm.__exit__(None, None, None)
        const_cm.__exit__(None, None, None)

    nc.compile()
    return nc


_built = {}


def _get_nc(N=4096, D=1024):
    key = (N, D)
    if key not in _built:
        _built[key] = build(N, D)
    return _built[key]


def _make_in_maps(inputs):
    q = np.ascontiguousarray(np.asarray(inputs["q"], dtype=np.float32))
    k = np.ascontiguousarray(np.asarray(inputs["k"], dtype=np.float32))
    v = np.ascontiguousarray(np.asarray(inputs["v"], dtype=np.float32))
    N, D = k.shape
    B = N // N_CORES
    shared = {}
    for nm in ("wq", "wk", "wv", "wo"):
        shared[nm + "_w"] = np.ascontiguousarray(
            np.asarray(inputs[nm + "_w"], dtype=np.float32))
        shared[nm + "_b"] = np.ascontiguousarray(
            np.asarray(inputs[nm + "_b"], dtype=np.float32))
    if N_LOCAL:
        shared["k_head"] = np.ascontiguousarray(k[:N_LOCAL * B])
    return [dict(shared,
                 q=q[c * B:(c + 1) * B],
                 k=np.ascontiguousarray(k[c * B:(c + 1) * B]),
                 v=np.ascontiguousarray(v[c * B:(c + 1) * B]))
            for c in range(N_CORES)], N, D


def kernel(**inputs):
    in_maps, N, D = _make_in_maps(inputs)
    nc = _get_nc(N, D)
    res = run_bass_kernel_spmd(nc, in_maps, core_ids=list(range(N_CORES)))
    x = np.concatenate([res.results[c]["x_out"] for c in range(N_CORES)], axis=0)
    attn = np.concatenate([res.results[c]["attn_out"] for c in range(N_CORES)],
                          axis=0)
    return (x, attn)


# revision 40
# speedup vs baseline: 1.0378x; 1.0106x over previous
"""Multi-head-attention (single-head, no scaling) Bass kernel for 8 trn2 cores.

v2: distributed K/V projections + AllGather.

Sharding: core c owns q rows [c*B, (c+1)*B) AND k/v rows [c*B, (c+1)*B).
Weights replicated. Each core:
  A)  wk transpose (PE, f32 -> rounded f32r copyback), own-k transpose,
      kpT_own = (wk @ k_own.T + b) [D, B] f32r -> DRAM -> AllGather (CC queue)
      wq transpose, qT, qpT (overlaps the AllGather)
  C)  wv (fp16) transpose, own-v cast+transpose, vp_own [B, D] fp16
      (+bias) -> DRAM -> AllGather; wo (fp16) transpose
  B1) for each 512-panel: DMA gathered kpT panel, DVE re-round to f32r,
      scores matmuls -> resident [B, N] f32
  B2) per 128-row tile: softmax (DVE/ACT, fused exp+rowsum), DMA attn out,
      cast fp16 + PE transpose -> attnT
  Dv) DMA gathered vp -> SBUF fp16
  D)  x1 = attn @ vp, transpose, x = x1 @ wo.T + b -> DMA out
"""

import numpy as np

_DIRECT_F32R = True

try:
    import concourse.bass  # noqa: F401
except ImportError:
    import sys
    for _p in ("/opt/trn_rl_repo", "/root/.axon_site/_ro/trn_rl_repo"):
        if _p not in sys.path:
            sys.path.append(_p)

import concourse.bacc as bacc
import concourse.mybir as mybir
from concourse.bass_utils import run_bass_kernel_spmd
from concourse.tile import TileContext
from concourse.masks import make_identity

P = 128
N_CORES = 8
N_LOCAL = 0   # head panels computed locally on every core

f32 = mybir.dt.float32
f32r = mybir.dt.float32r
f16 = mybir.dt.float16
EXP = mybir.ActivationFunctionType.Exp
COPY = mybir.ActivationFunctionType.Copy
IDENT = mybir.ActivationFunctionType.Identity


def build(N=4096, D=1024, n_local=N_LOCAL):
    B = N // N_CORES            # q/k/v rows per core
    DC = D // P                 # contraction chunks (d)
    IT = B // P                 # 128-row tiles per core slice
    PW = B                      # scores panel width == per-core slice
    LPAN = N // PW
    LT = N // P
    FS = min(512, D)
    MS = D // FS
    LHEAD = n_local * B         # head rows computed locally on every core

    nc = bacc.Bacc("TRN2", target_bir_lowering=False, debug=False,
                   num_devices=N_CORES)

    q_d = nc.dram_tensor("q", [B, D], f32, kind="ExternalInput")
    k_d = nc.dram_tensor("k", [B, D], f32, kind="ExternalInput")
    v_d = nc.dram_tensor("v", [B, D], f32, kind="ExternalInput")
    if n_local:
        kh_d = nc.dram_tensor("k_head", [LHEAD, D], f32, kind="ExternalInput")
    w_d = {}
    b_d = {}
    for nm in ("wq", "wk", "wv", "wo"):
        w_d[nm] = nc.dram_tensor(nm + "_w", [D, D], f32, kind="ExternalInput")
        b_d[nm] = nc.dram_tensor(nm + "_b", [D], f32, kind="ExternalInput")
    x_out = nc.dram_tensor("x_out", [B, D], f32, kind="ExternalOutput")
    a_out = nc.dram_tensor("attn_out", [B, N], f32, kind="ExternalOutput")

    with TileContext(nc) as tc:
        const_cm = tc.tile_pool(name="const", bufs=1)
        const = const_cm.__enter__()
        dram_cm = tc.tile_pool(name="dram", bufs=1, space="DRAM")
        dram = dram_cm.__enter__()

        kp_in = dram.tile([D, B], f32, tag="kp_in")
        JH = D // 2             # feature-half split of the kp gather
        kp_all_j = [dram.tile([N_CORES, JH, B], f32, tag=f"kp_all{j}",
                              name=f"kp_all{j}") for j in range(2)]
        vp_in = dram.tile([B, D], f16, tag="vp_in")
        vp_all = dram.tile([N_CORES, B, D], f16, tag="vp_all")
        warm_in = dram.tile([1, 32], f32, tag="warm_in")
        warm_out = dram.tile([N_CORES, 32], f32, tag="warm_out")

        # tiny warm-up collective: absorbs comm-init/rendezvous cost while
        # the input DMAs and weight transposes run
        warm_sb = const.tile([1, 32], f32, tag="warm")
        nc.gpsimd.memset(warm_sb[:], 0.0)
        nc.gpsimd.dma_start(warm_in[:], warm_sb[:])
        nc.gpsimd.collective_compute(
            "AllGather", mybir.AluOpType.bypass,
            replica_groups=[list(range(N_CORES))],
            ins=[warm_in[:].opt()], outs=[warm_out[:].opt()])

        ident32 = const.tile([P, P], f32, tag="id32")
        make_identity(nc, ident32[:])
        ident16 = const.tile([P, P], f16, tag="id16")
        make_identity(nc, ident16[:])

        qb_col = const.tile([P, DC], f32, tag="qbcol")
        kb_col = const.tile([P, DC], f32, tag="kbcol")
        for jt in range(DC):
            nc.sync.dma_start(qb_col[:, jt:jt + 1],
                              b_d["wq"][jt * P:(jt + 1) * P])
            nc.sync.dma_start(kb_col[:, jt:jt + 1],
                              b_d["wk"][jt * P:(jt + 1) * P])
        vb_bc = const.tile([P, D], f16, tag="vbbc")
        ob_bc = const.tile([P, D], f16, tag="obbc")
        with tc.tile_pool(name="btmp", bufs=2) as btmp:
            for bc_t, bnm in ((vb_bc, "wv"), (ob_bc, "wo")):
                tmp = btmp.tile([P, D], f32, tag="btmp")
                nc.sync.dma_start(
                    tmp[:],
                    b_d[bnm].rearrange("(o d) -> o d", o=1).partition_broadcast(P))
                nc.vector.tensor_copy(bc_t[:], tmp[:])

        def transpose_weight(nm, wT, sb, ps, rnd):
            """DMA weight natural, PE-transpose 128x128 blocks into wT."""
            for jt in range(DC):
                wnat = sb.tile([P, D], f32, tag="wnat")
                nc.sync.dma_start(wnat[:], w_d[nm][jt * P:(jt + 1) * P, :])
                for dt in range(DC):
                    pt = ps.tile([P, P], f32, tag="tpw")
                    nc.tensor.transpose(pt[:], wnat[:, dt * P:(dt + 1) * P],
                                        ident32[:])
                    dst = wT[:, dt, jt * P:(jt + 1) * P]
                    if rnd is f32r:
                        dst = dst.bitcast(f32r)
                    # balance copybacks across ACT and DVE
                    if dt % 2 == 0:
                        nc.scalar.activation(dst, pt[:], COPY)
                    else:
                        nc.vector.tensor_copy(dst, pt[:])

        def transpose_rows(src_d, row0, rows, dst, sb, ps, dt_out):
            """DMA rows [row0, row0+rows) of src_d, transpose into
            dst [P, DC, rows]."""
            for rt in range(rows // P):
                nat = sb.tile([P, D], f32, tag="wnat")
                nc.sync.dma_start(
                    nat[:], src_d[row0 + rt * P:row0 + (rt + 1) * P, :])
                if dt_out is f16:
                    c16 = sb.tile([P, D], f16, tag="nat16")
                    nc.vector.tensor_copy(c16[:], nat[:])
                    for dt in range(DC):
                        pt = ps.tile([P, P], f16, tag="tp16")
                        nc.tensor.transpose(pt[:], c16[:, dt * P:(dt + 1) * P],
                                            ident16[:])
                        nc.vector.tensor_copy(dst[:, dt, rt * P:(rt + 1) * P],
                                              pt[:])
                else:
                    for dt in range(DC):
                        pt = ps.tile([P, P], f32, tag="tpw")
                        nc.tensor.transpose(pt[:], nat[:, dt * P:(dt + 1) * P],
                                            ident32[:])
                        d2 = dst[:, dt, rt * P:(rt + 1) * P].bitcast(f32r)
                        if dt % 2 == 0:
                            nc.vector.tensor_copy(d2, pt[:])
                        else:
                            nc.scalar.activation(d2, pt[:], COPY)

        def project(wT, xT, out_cb, ps2, nmm=None):
            """out[jt] = (wT.T @ xT) per jt chunk; out_cb(jt, psum)."""
            width = xT.shape[2] if nmm is None else nmm
            for jt in range(DC):
                pj = ps2.tile([P, width], f32, tag="pq", name=f"pj_{jt}")
                for dt in range(DC):
                    nc.tensor.matmul(pj[:],
                                     wT[:, dt, jt * P:(jt + 1) * P].bitcast(f32r),
                                     xT[:, dt, :].bitcast(f32r),
                                     start=(dt == 0), stop=(dt == DC - 1))
                out_cb(jt, pj)

        def scores_panel(col0, kpSrc, b_ps3):
            """scores[:, :, col0:col0+width] for all IT row-tiles."""
            width = kpSrc.shape[2]
            for it in range(IT):
                psc = b_ps3.tile([P, width], f32, tag="ps", name=f"psc_{it}")
                for jt in range(DC):
                    nc.tensor.matmul(
                        psc[:],
                        qpT[:, jt, it * P:(it + 1) * P].bitcast(f32r),
                        kpSrc[:, jt, :].bitcast(f32r),
                        start=(jt == 0), stop=(jt == DC - 1))
                nc.vector.tensor_copy(
                    scoresR[:, it, col0:col0 + width], psc[:])

        # ---- phase A: kpT_own -> AllGather; qpT; local head panels ---
        poolA_cm = tc.tile_pool(name="qpT", bufs=1)   # qpT: A..B1 end
        poolA = poolA_cm.__enter__()
        qpT = poolA.tile([P, DC, B], f32, tag="qpT")

        a_w_cm = tc.tile_pool(name="a_w", bufs=1)     # wkT: A..B1a end
        a_w = a_w_cm.__enter__()
        wkT = a_w.tile([P, DC, D], f32, tag="wkT")

        with (
            tc.tile_pool(name="a_q", bufs=1) as a_q,
            tc.tile_pool(name="a_sb", bufs=3) as a_sb,
            tc.tile_pool(name="a_ps", bufs=3, space="PSUM") as a_ps,
            tc.tile_pool(name="a_ps2", bufs=2, space="PSUM") as a_ps2,
        ):
            # --- k path first so the AllGather launches early ---
            transpose_weight("wk", wkT, a_sb, a_ps, f32r)
            kTo = a_q.tile([P, DC, B], f32, tag="kTo")
            transpose_rows(k_d, 0, B, kTo, a_sb, a_ps, f32r)
            kpTo = a_q.tile([P, DC, B], f32, tag="kpTo")

            def kp_cb(jt, pj):
                nc.scalar.activation(kpTo[:, jt, :].bitcast(f32r), pj[:], IDENT,
                                     bias=kb_col[:, jt:jt + 1])
                nc.sync.dma_start(kp_in[jt * P:(jt + 1) * P, :], kpTo[:, jt, :])
                if (jt + 1) * P == JH or (jt + 1) * P == D:
                    j = 0 if (jt + 1) * P == JH else 1
                    nc.gpsimd.collective_compute(
                        "AllGather", mybir.AluOpType.bypass,
                        replica_groups=[list(range(N_CORES))],
                        ins=[kp_in[j * JH:(j + 1) * JH, :].opt()],
                        outs=[kp_all_j[j][:].opt()])
            project(wkT, kTo, kp_cb, a_ps2)

            # --- q path (overlaps the gather) ---
            wqT = a_q.tile([P, DC, D], f32, tag="wqT")
            transpose_weight("wq", wqT, a_sb, a_ps, f32r)
            qT = a_q.tile([P, DC, B], f32, tag="qT")
            transpose_rows(q_d, 0, B, qT, a_sb, a_ps, f32r)

            def qp_cb(jt, pj):
                nc.scalar.activation(qpT[:, jt, :].bitcast(f32r), pj[:], IDENT,
                                     bias=qb_col[:, jt:jt + 1])
            project(wqT, qT, qp_cb, a_ps2)

        poolC_cm = tc.tile_pool(name="woT", bufs=1, side="right")   # woT: ..D
        poolC = poolC_cm.__enter__()
        woT = poolC.tile([P, DC, D], f16, tag="woT")

        # ---- B1a: local head panels (fill the AllGather window) ------
        poolB_cm = tc.tile_pool(name="scores", bufs=1, side="right")
        poolB = poolB_cm.__enter__()
        scoresR = poolB.tile([P, IT, N], f32, tag="scores")

        if n_local:
            with (
                tc.tile_pool(name="l_sb", bufs=1) as l_sb,
                tc.tile_pool(name="l_k", bufs=3) as l_k,
                tc.tile_pool(name="l_ps", bufs=2, space="PSUM") as l_ps,
                tc.tile_pool(name="l_ps2", bufs=2, space="PSUM") as l_ps2,
                tc.tile_pool(name="l_ps3", bufs=4, space="PSUM") as l_ps3,
            ):
                for lp in range(n_local):
                    kTl = l_sb.tile([P, DC, B], f32, tag="kTl")
                    transpose_rows(kh_d, lp * B, B, kTl, l_k, l_ps, f32r)
                    kpTl = l_sb.tile([P, DC, B], f32, tag="kpTl")

                    def kpl_cb(jt, pj, kpTl=kpTl):
                        nc.scalar.activation(kpTl[:, jt, :].bitcast(f32r),
                                             pj[:], IDENT,
                                             bias=kb_col[:, jt:jt + 1])
                    project(wkT, kTl, kpl_cb, l_ps2)
                    scores_panel(lp * PW, kpTl, l_ps3)
        a_w_cm.__exit__(None, None, None)   # free wkT

        # ---- phase C: vp_own -> AllGather; woT -----------------------
        with (
            tc.tile_pool(name="c_w", bufs=1) as c_w,
            tc.tile_pool(name="c_sb", bufs=3) as c_sb,
            tc.tile_pool(name="c_ps", bufs=3, space="PSUM") as c_ps,
            tc.tile_pool(name="c_ps2", bufs=2, space="PSUM") as c_ps2,
        ):
            wvT = c_w.tile([P, DC, D], f16, tag="wvT")
            transpose_weight("wv", wvT, c_sb, c_ps, f16)
            vTo = c_w.tile([P, DC, B], f16, tag="vTo")
            transpose_rows(v_d, 0, B, vTo, c_sb, c_ps, f16)
            vpo = c_w.tile([P, IT, D], f16, tag="vpo")
            for lc in range(IT):
                for ms in range(MS):
                    pv = c_ps2.tile([P, FS], f32, tag="pv")
                    for dt in range(DC):
                        nc.tensor.matmul(pv[:], vTo[:, dt, lc * P:(lc + 1) * P],
                                         wvT[:, dt, ms * FS:(ms + 1) * FS],
                                         start=(dt == 0), stop=(dt == DC - 1))
                    nc.vector.tensor_add(vpo[:, lc, ms * FS:(ms + 1) * FS],
                                         pv[:], vb_bc[:, ms * FS:(ms + 1) * FS])
                nc.sync.dma_start(vp_in[lc * P:(lc + 1) * P, :], vpo[:, lc, :])
            nc.gpsimd.collective_compute(
                "AllGather", mybir.AluOpType.bypass,
                replica_groups=[list(range(N_CORES))],
                ins=[vp_in[:].opt()], outs=[vp_all[:].opt()])

            # wo transposes fill the gather wait
            transpose_weight("wo", woT, c_sb, c_ps, f16)

        # ---- phase B1b: gathered kpT panels -> scores ----------------
        JC = JH // P            # jt chunks per feature-half
        with (
            tc.tile_pool(name="b_sb", bufs=2) as b_sb,
            tc.tile_pool(name="b_ps3", bufs=1, space="PSUM") as b_ps3,
        ):
            panels = list(range(n_local, LPAN))
            for w in range(0, len(panels), 2):
                wave = panels[w:w + 2]
                kpPs, pscs = {}, {}
                # pass 1: feature-half 0 for both panels in the wave
                for wi, lp in enumerate(wave):
                    kpP = b_sb.tile([P, DC, PW], f32, tag=f"kpP{wi}",
                                    name=f"kpP{wi}")
                    kpPs[lp] = kpP
                    for jt in range(JC):
                        nc.sync.dma_start(
                            kpP[:, jt, :].bitcast(f32r),
                            kp_all_j[0][lp, jt * P:(jt + 1) * P, :].bitcast(f32r))
                    for it in range(IT):
                        psc = b_ps3.tile([P, PW], f32, tag=f"ps{wi}_{it}",
                                         name=f"psc{wi}_{it}")
                        pscs[(lp, it)] = psc
                        for jt in range(JC):
                            nc.tensor.matmul(
                                psc[:],
                                qpT[:, jt, it * P:(it + 1) * P].bitcast(f32r),
                                kpP[:, jt, :].bitcast(f32r),
                                start=(jt == 0), stop=False)
                # pass 2: feature-half 1, finish accumulation + copy back
                for lp in wave:
                    kpP = kpPs[lp]
                    for jt in range(JC, DC):
                        nc.scalar.dma_start(
                            kpP[:, jt, :].bitcast(f32r),
                            kp_all_j[1][lp, (jt - JC) * P:(jt - JC + 1) * P, :]
                            .bitcast(f32r))
                    for it in range(IT):
                        psc = pscs[(lp, it)]
                        for jt in range(JC, DC):
                            nc.tensor.matmul(
                                psc[:],
                                qpT[:, jt, it * P:(it + 1) * P].bitcast(f32r),
                                kpP[:, jt, :].bitcast(f32r),
                                start=False, stop=(jt == DC - 1))
                        dst2 = scoresR[:, it, lp * PW:(lp + 1) * PW]
                        if it % 2 == 0:
                            nc.vector.tensor_copy(dst2, psc[:])
                        else:
                            nc.scalar.activation(dst2, psc[:], COPY)
        poolA_cm.__exit__(None, None, None)   # free qpT

        # ---- merged B2 + D: softmax, attnT, x1, out ------------------
        poolBD_cm = tc.tile_pool(name="attnT", bufs=1)
        poolBD = poolBD_cm.__enter__()
        attnT = poolBD.tile([P, LT, B], f16, tag="attnT")
        vp = poolBD.tile([P, LT, D], f16, tag="vp")

        # vp: DMA gathered [N_CORES, B, D] -> [P, LT, D]
        for lc in range(LT):
            c, r = divmod(lc, IT)
            nc.gpsimd.dma_start(vp[:, lc, :], vp_all[c, r * P:(r + 1) * P, :])

        with (
            tc.tile_pool(name="s_sb", bufs=2) as s_sb,
            tc.tile_pool(name="d_sb", bufs=1) as d_sb,
            tc.tile_pool(name="s_ps", bufs=4, space="PSUM") as s_ps,
            tc.tile_pool(name="d_ps", bufs=2, space="PSUM") as d_ps,
            tc.tile_pool(name="d_ps2", bufs=2, space="PSUM") as d_ps2,
        ):
            for it in range(IT):
                sc = scoresR[:, it, :]
                nmax = s_sb.tile([P, 1], f32, tag="nmax")
                nc.vector.reduce_max(nmax[:], sc, axis=mybir.AxisListType.X,
                                     negate=True)
                zsum = s_sb.tile([P, 1], f32, tag="zsum")
                nc.scalar.activation(sc, sc, EXP, bias=nmax[:], scale=1.0,
                                     accum_out=zsum[:])
                zinv = s_sb.tile([P, 1], f32, tag="zinv")
                nc.vector.reciprocal(zinv[:], zsum[:])
                # ACT fuses normalize+fp16 cast (scale=1/Z) feeding the
                # transposes; the DVE f32 normalize for the attn output runs
                # in parallel off the critical path.
                for lg in range(N // FS):
                    ab = s_sb.tile([P, FS], f16, tag="abf")
                    nc.scalar.activation(ab[:], sc[:, lg * FS:(lg + 1) * FS],
                                         COPY, scale=zinv[:])
                    for li in range(FS // P):
                        lt = lg * (FS // P) + li
                        pt = s_ps.tile([P, P], f16, tag="tp16")
                        nc.tensor.transpose(pt[:], ab[:, li * P:(li + 1) * P],
                                            ident16[:])
                        dst = attnT[:, lt, it * P:(it + 1) * P]
                        if lt % 2 == 0:
                            nc.vector.tensor_copy(dst, pt[:])
                        else:
                            nc.scalar.activation(dst, pt[:], COPY)
                nc.vector.tensor_scalar_mul(sc, sc, zinv[:])
                nc.sync.dma_start(a_out[it * P:(it + 1) * P, :], sc)

                x1b = d_sb.tile([P, D], f16, tag="x1b")
                for ms in range(MS):
                    px = d_ps.tile([P, FS], f32, tag="px")
                    for lt in range(LT):
                        nc.tensor.matmul(px[:], attnT[:, lt, it * P:(it + 1) * P],
                                         vp[:, lt, ms * FS:(ms + 1) * FS],
                                         start=(lt == 0), stop=(lt == LT - 1))
                    nc.scalar.activation(x1b[:, ms * FS:(ms + 1) * FS], px[:],
                                         COPY)
                x1T = d_sb.tile([P, DC, P], f16, tag="x1T")
                for mt in range(DC):
                    pt = s_ps.tile([P, P], f16, tag="tp16")
                    nc.tensor.transpose(pt[:], x1b[:, mt * P:(mt + 1) * P],
                                        ident16[:])
                    nc.vector.tensor_copy(x1T[:, mt, :], pt[:])
                xo = d_sb.tile([P, D], f32, tag="xo")
                for cs in range(MS):
                    pxo = d_ps2.tile([P, FS], f32, tag="pxo")
                    for mt in range(DC):
                        nc.tensor.matmul(pxo[:], x1T[:, mt, :],
                                         woT[:, mt, cs * FS:(cs + 1) * FS],
                                         start=(mt == 0), stop=(mt == DC - 1))
                    nc.vector.tensor_add(xo[:, cs * FS:(cs + 1) * FS], pxo[:],
                                         ob_bc[:, cs * FS:(cs + 1) * FS])
                nc.scalar.dma_start(x_out[it * P:(it + 1) * P, :], xo[:])
        poolB_cm.__exit__(None, None, None)   # free scoresR

        poolBD_cm.__exit__(None, None, None)
        poolC_cm.__exit__(None, None, None)
        dram_cm.__exit__(None, None, None)
        const_cm.__exit__(None, None, None)

    nc.compile()
    return nc


_built = {}


def _get_nc(N=4096, D=1024):
    key = (N, D)
    if key not in _built:
        _built[key] = build(N, D)
    return _built[key]


def _make_in_maps(inputs):
    q = np.ascontiguousarray(np.asarray(inputs["q"], dtype=np.float32))
    k = np.ascontiguousarray(np.asarray(inputs["k"], dtype=np.float32))
    v = np.ascontiguousarray(np.asarray(inputs["v"], dtype=np.float32))
    N, D = k.shape
    B = N // N_CORES
    shared = {}
    for nm in ("wq", "wk", "wv", "wo"):
        shared[nm + "_w"] = np.ascontiguousarray(
            np.asarray(inputs[nm + "_w"], dtype=np.float32))
        shared[nm + "_b"] = np.ascontiguousarray(
            np.asarray(inputs[nm + "_b"], dtype=np.float32))
    if N_LOCAL:
        shared["k_head"] = np.ascontiguousarray(k[:N_LOCAL * B])
    return [dict(shared,
                 q=q[c * B:(c + 1) * B],
                 k=np.ascontiguousarray(k[c * B:(c + 1) * B]),
                 v=np.ascontiguousarray(v[c * B:(c + 1) * B]))
            for c in range(N_CORES)], N, D


def kernel(**inputs):
    in_maps, N, D = _make_in_maps(inputs)
    nc = _get_nc(N, D)
    res = run_bass_kernel_spmd(nc, in_maps, core_ids=list(range(N_CORES)))
    x = np.concatenate([res.results[c]["x_out"] for c in range(N_CORES)], axis=0)
    attn = np.concatenate([res.results[c]["attn_out"] for c in range(N_CORES)],
                          axis=0)
    return (x, attn)
